# revision 36
# baseline (speedup 1.0000x reference)
"""Trainium2 Bass kernel: causal self-attention (GQA + RoPE) for
B=1, T=2048, C=2048, H=16 query heads, HKV=4 KV heads, D=128.

Sharding: tensor-parallel over heads across 8 NeuronCores. Core m computes
query heads {2m, 2m+1} and the single KV head (m//2) those heads attend to,
plus the o_proj partial product for its 256 input columns. The host sums the
8 partial outputs (the TP all-reduce).

Device-side layout (per core) is transposed so every matmul has its
contraction on the partition axis with no on-chip transposes of activations:
  qT[h]  = (Wq_h @ x.T + bq)/sqrt(D)   [D=128 part, T free]   (ACT eviction)
  kT     =  Wk_g @ x.T + bk            [128, T]
  vT     =  Wv_g @ x.T + bv            [128, T] -> PE-transposed to v [T,128]
  RoPE on qT/kT via sign-folded sin + partition-swap DMA copies.
  ST     = k'T.T @ q'T                 [tk 128-part, tq 512-free] per head
  P      = exp(ST) (no max subtraction: logits are bounded), causal mask via
           a triangular [128,128] multiply on diagonal tiles; fully-masked
           column ranges are skipped in the matmuls entirely.
  sums   = ones128.T @ P  (column sums broadcast across partitions, in PSUM)
  yT     = matmul(lhsT=v[tk,128], rhs=P) accumulated over tk
  y'T    = yT * reciprocal(sums)
  out    = y'T.T @ WoT (partial; host sums over cores)

Matmuls run as float32r (fp32 bits, reduced-precision PE mode, 1 cyc/row for
free dim >= 256 vs 4 for fp32). The BIR verifier requires every producer of
an fp32r matmul operand to emit fp32r, so those buffers are declared fp32r
end-to-end (DRAM and SBUF); engine output-casts do the advisory rounding.
"""

import math
import numpy as np
from contextlib import ExitStack

import concourse.bass as bass
import concourse.bacc as bacc
import concourse.tile as tile
from concourse import mybir
from concourse.bass_utils import run_bass_kernel_spmd
from concourse.masks import make_identity

B, T, C = 1, 2048, 2048
H, HKV = 16, 4
D = 128
NCORES = 8
HL = H // NCORES          # query heads per core
TQ = 512                  # query tile width (one fp32 PSUM bank)
NT = T // TQ              # 4 query tiles
NK = T // D               # 16 key tiles
NCT = C // 128            # 16 contraction tiles over the model dim
F32 = mybir.dt.float32
F32R = mybir.dt.float32r
Id = mybir.ActivationFunctionType.Identity
Exp = mybir.ActivationFunctionType.Exp

_CACHE: dict = {}


def _build():
    nc = bacc.Bacc(None, target_bir_lowering=False, debug=False)
    scale = 1.0 / math.sqrt(D)
    with tile.TileContext(nc) as tc, ExitStack() as ctx:
        dram = ctx.enter_context(tc.tile_pool(name="dram", bufs=1, space="DRAM"))

        def din(name, shape, dt=F32R):
            return dram.tile(shape, dt, kind="ExternalInput", name=name,
                             uniquify=False)

        xt_d = din("xt", [C, T])          # x[0].T
        # weights arrive host-pre-tiled to the SBUF layout [128, ...] so
        # each loads in one dma_start with 8-16KB per-partition descriptors
        wq_d = din("wq", [128, NCT * HL * D])
        wk_d = din("wk", [128, NCT * D])
        wv_d = din("wv", [128, NCT * D])
        wo_d = din("wo", [128, HL * C])
        tri_d = din("tri", [D, D])        # tri[i,j] = 1.0 if i<=j else 0.0
        one_d = din("ones", [D, D])
        bq_d = din("bq", [D, HL], F32)    # pre-scaled by 1/sqrt(D)
        bk_d = din("bk", [D, 1], F32)
        bv_d = din("bv", [D, 1], F32)
        cos_d = din("cost", [D, T], F32)  # cos[0].T
        sin_d = din("sins", [D, T], F32)  # sin[0].T with rows 0:64 negated
        out_d = dram.tile([T, C], mybir.dt.bfloat16, kind="ExternalOutput",
                          name="out", uniquify=False)

        const = ctx.enter_context(tc.tile_pool(name="const", bufs=1))
        wq_s = const.tile([128, NCT * HL * D], F32R, name="wq_s")
        wk_s = const.tile([128, NCT * D], F32R, name="wk_s")
        wv_s = const.tile([128, NCT * D], F32R, name="wv_s")
        # Weight loads go on the GpSimd SWDGE queue: the Sync sequencer
        # issues each dma_start serially (~700ns apiece), and it is needed
        # for the latency-critical x/out streams.
        nc.gpsimd.dma_start(out=wk_s[:], in_=wk_d[:])
        nc.gpsimd.dma_start(out=wv_s[:], in_=wv_d[:])
        nc.gpsimd.dma_start(out=wq_s[:], in_=wq_d[:])
        bq_s = const.tile([128, HL], F32, name="bq_s")
        bk_s = const.tile([128, 1], F32, name="bk_s")
        bv_s = const.tile([128, 1], F32, name="bv_s")
        cos_s = const.tile([128, T], F32, name="cos_s")
        sin_s = const.tile([128, T], F32, name="sin_s")
        tri_s = const.tile([128, 128], F32R, name="tri_s")
        ones_s = const.tile([128, 128], F32R, name="ones_s")
        ident_s = const.tile([128, 128], F32, name="ident_s")
        for dst, src in ((bq_s, bq_d), (bk_s, bk_d), (bv_s, bv_d),
                         (cos_s, cos_d), (sin_s, sin_d), (tri_s, tri_d),
                         (ones_s, one_d)):
            nc.sync.dma_start(out=dst[:], in_=src[:])
        make_identity(nc, ident_s[:])
        wo_s = const.tile([128, HL * C], F32R, name="wo_s")

        act = ctx.enter_context(tc.tile_pool(name="act", bufs=1))
        qr = [act.tile([128, T], F32R, name=f"qr{h}_s") for h in range(HL)]
        kr_s = act.tile([128, T], F32R, name="kr_s")
        vT_s = act.tile([128, T], F32, name="vT_s")
        v_s = act.tile([128, NK * D], F32R, name="v_s")
        ys = [act.tile([128, T], F32R, name=f"y{h}_s") for h in range(HL)]

        xpool = ctx.enter_context(tc.tile_pool(name="xpool", bufs=6))
        prepool = ctx.enter_context(tc.tile_pool(name="prepool", bufs=2))
        rpool = ctx.enter_context(tc.tile_pool(name="rpool", bufs=2))
        ppool = ctx.enter_context(tc.tile_pool(name="ppool", bufs=4))
        # wide out tiles recycle only after their DMA fully completes
        # (~2us HBM write-completion each), so keep 6 in flight
        opool = ctx.enter_context(tc.tile_pool(name="opool", bufs=6))

        def rope(dst, pre, t):
            """dst = pre*cos + rot_half(pre)*sin on columns [t*TQ, (t+1)*TQ).

            dst is fp32r; the final add casts. All DVE inputs stay fp32.
            The partition-swap copies ride the GpSimd SWDGE queue.
            """
            sl = bass.ts(t, TQ)
            rot = rpool.tile([128, TQ], F32, tag="rot")
            nc.gpsimd.dma_start(out=rot[0:64, :], in_=pre[64:128, :])
            nc.gpsimd.dma_start(out=rot[64:128, :], in_=pre[0:64, :])
            nc.vector.tensor_mul(rot[:], rot[:], sin_s[:, sl])
            tmp = rpool.tile([128, TQ], F32, tag="rtmp")
            nc.vector.tensor_mul(tmp[:], pre[:], cos_s[:, sl])
            nc.vector.tensor_add(dst, tmp[:], rot[:])

        # ---- interleaved projection-pair / attention blocks --------------
        # t-pair blocks: one [128, 1024] x load feeds both halves' psum
        # accumulators (8 banks), halving dma_start count and giving 4KB
        # DMA lines. Attention for tq in {2tp, 2tp+1} only needs
        # projections <= the pair, so each pair's attention+o_proj runs
        # right after it: PE never crosses an idle phase boundary and the
        # output DMA stream starts halfway through the kernel.
        def proj_pair(tp, pa):
            ps = []  # [half][q0, q1, k, v]
            for half in range(2):
                ps.append([pa.tile([128, TQ], F32, tag=f"pp{half}{j}",
                                   name=f"pp{half}{j}")
                           for j in range(4)])
            for c in range(NCT):
                xt = xpool.tile([128, 2 * TQ], F32R, tag="x")
                nc.sync.dma_start(
                    out=xt[:],
                    in_=xt_d[bass.ts(c, 128), tp * 2 * TQ:(tp + 1) * 2 * TQ])
                st, sp = (c == 0), (c == NCT - 1)
                base = c * HL * D
                for half in range(2):
                    xh = xt[:, bass.ts(half, TQ)]
                    nc.tensor.matmul(ps[half][0][:], wq_s[:, base:base + D],
                                     xh, start=st, stop=sp)
                    nc.tensor.matmul(ps[half][1][:],
                                     wq_s[:, base + D:base + 2 * D],
                                     xh, start=st, stop=sp)
                    nc.tensor.matmul(ps[half][2][:], wk_s[:, bass.ts(c, D)],
                                     xh, start=st, stop=sp)
                    nc.tensor.matmul(ps[half][3][:], wv_s[:, bass.ts(c, D)],
                                     xh, start=st, stop=sp)
            # PSUM->SBUF evictions with fused bias (and 1/sqrt(D) on q),
            # split across ACT and DVE so the pair-boundary drains fast
            for half in range(2):
                t = 2 * tp + half
                sl = bass.ts(t, TQ)
                pre0 = prepool.tile([128, TQ], F32, tag="pre0")
                pre1 = prepool.tile([128, TQ], F32, tag="pre1")
                prek = prepool.tile([128, TQ], F32, tag="prek")
                nc.vector.tensor_scalar(pre0[:], ps[half][0][:], scale,
                                        bq_s[:, 0:1], mybir.AluOpType.mult,
                                        mybir.AluOpType.add)
                nc.vector.tensor_scalar(pre1[:], ps[half][1][:], scale,
                                        bq_s[:, 1:2], mybir.AluOpType.mult,
                                        mybir.AluOpType.add)
                nc.scalar.activation(prek[:], ps[half][2][:], Id,
                                     bias=bk_s[:, 0:1])
                nc.scalar.activation(vT_s[:, sl], ps[half][3][:], Id,
                                     bias=bv_s[:, 0:1])
                rope(qr[0][:, sl], pre0, t)
                rope(qr[1][:, sl], pre1, t)
                rope(kr_s[:, sl], prek, t)

        def vtrans(tp, vt):
            # transpose the pair's 8 new v tiles to natural layout; evict on
            # ACT (idle during phase A while DVE runs RoPE)
            for tk in range(8 * tp, 8 * tp + 8):
                vtp = vt.tile([128, D], F32, tag="vtp", name="vtp")
                nc.tensor.transpose(vtp[:], vT_s[:, bass.ts(tk, D)],
                                    ident_s[:])
                nc.scalar.copy(v_s[:, bass.ts(tk, D)], vtp[:])

        def attn_block(tq, pb):
            ntk = 4 * tq + 4
            if True:
                for h in range(HL):
                    sump = pb.tile([128, TQ], F32, tag="sum", bufs=1)
                    yp = pb.tile([128, TQ], F32, tag="yav")
                    # software-pipelined: score matmul+exp one tile ahead of
                    # the consuming sum/AV matmuls so PE never waits on ACT
                    pts = {}
                    for tk in range(ntk + 1):
                        if tk < ntk:
                            r = max(tk * D - tq * TQ, 0)  # masked col prefix
                            sp_ = pb.tile([128, TQ], F32, tag="s")
                            nc.tensor.matmul(
                                sp_[:, r:], kr_s[:, bass.ts(tk, D)],
                                qr[h][:, tq * TQ + r:(tq + 1) * TQ],
                                start=True, stop=True)
                            pt = ppool.tile([128, TQ], F32R, tag="p")
                            nc.scalar.activation(pt[:, r:], sp_[:, r:], Exp)
                            if tk * D >= tq * TQ:  # diagonal: triangular mask
                                # on GpSimd: idle all of phase B, and this
                                # keeps the in-order DVE queue short
                                nc.gpsimd.tensor_mul(pt[:, r:r + D],
                                                     pt[:, r:r + D], tri_s[:])
                            pts[tk] = (pt, r)
                        if tk >= 1:
                            pt, r = pts.pop(tk - 1)
                            st, sp2 = (tk - 1 == 0), (tk - 1 == ntk - 1)
                            nc.tensor.matmul(sump[:, r:], ones_s[:],
                                             pt[:, r:], start=st, stop=sp2)
                            nc.tensor.matmul(yp[:, r:],
                                             v_s[:, bass.ts(tk - 1, D)],
                                             pt[:, r:], start=st, stop=sp2)
                    rec = rpool.tile([128, TQ], F32, tag="rec")
                    nc.vector.reciprocal_approx_fast(rec[:], sump[:])
                    nc.vector.tensor_mul(ys[h][:, bass.ts(tq, TQ)], yp[:],
                                         rec[:])
                # o_proj for the 4 row-tiles of this tq block; evict the 4
                # column tiles into one wide bf16 tile -> DMA with 4KB lines
                # per output row-block. During the steady state ACT is the
                # exp engine, so evictions ride DVE; on the last block ACT
                # is free, so alternate engines and split the DMA in halves
                # to drain the tail faster.
                last = tq == NT - 1
                for tt in range(4):
                    t = 4 * tq + tt
                    wide = opool.tile([128, C], mybir.dt.bfloat16, tag="oev")
                    for n in range(NT):
                        op_ = pb.tile([128, TQ], F32, tag="o", bufs=3)
                        for h in range(HL):
                            nc.tensor.matmul(
                                op_[:], ys[h][:, bass.ts(t, D)],
                                wo_s[:, h * C + n * TQ:h * C + (n + 1) * TQ],
                                start=(h == 0), stop=(h == HL - 1))
                        if (last and n % 2 == 0) or (not last and n == 0):
                            nc.scalar.copy(wide[:, bass.ts(n, TQ)], op_[:])
                        else:
                            nc.vector.tensor_copy(wide[:, bass.ts(n, TQ)],
                                                  op_[:])
                        if n == 1:  # first half flies while second evicts
                            nc.sync.dma_start(
                                out=out_d[bass.ts(t, D), 0:2 * TQ],
                                in_=wide[:, 0:2 * TQ])
                    nc.sync.dma_start(out=out_d[bass.ts(t, D), 2 * TQ:C],
                                      in_=wide[:, 2 * TQ:C])

        with tc.tile_pool(name="pa_psum", bufs=1, space="PSUM") as pa:
            for tp in range(NT // 2):
                proj_pair(tp, pa)
        with tc.tile_pool(name="vt_psum", bufs=4, space="PSUM") as vt:
            for tp in range(NT // 2):
                vtrans(tp, vt)
        nc.gpsimd.dma_start(out=wo_s[:], in_=wo_d[:])
        with tc.tile_pool(name="pb_psum", bufs=2, space="PSUM") as pb:
            for tq in range(NT):
                attn_block(tq, pb)
    nc.compile()
    return nc


def _get_nc():
    if "nc" not in _CACHE:
        _CACHE["nc"] = _build()
    return _CACHE["nc"]


def _prep_inputs(x, cos, sin, Wq, bq, Wk, bk, Wv, bv, Wo):
    f = np.float32
    xT = np.ascontiguousarray(x[0].T, dtype=f)
    cosT = np.ascontiguousarray(cos[0].T, dtype=f)
    sinT = np.ascontiguousarray(sin[0].T, dtype=f)
    sins = np.concatenate([-sinT[:64], sinT[64:]], axis=0)
    sins = np.ascontiguousarray(sins, dtype=f)
    idx = np.arange(D)
    tri = (idx[:, None] <= idx[None, :]).astype(f)
    ones = np.ones((D, D), dtype=f)
    scale = np.float32(1.0 / math.sqrt(D))
    in_maps = []
    def ptile(a):
        """[K*128, N] -> partition-major [128, K*N] matching the SBUF tiles."""
        k = a.shape[0] // 128
        return np.ascontiguousarray(
            a.reshape(k, 128, a.shape[1]).transpose(1, 0, 2).reshape(128, -1),
            dtype=f)

    for m in range(NCORES):
        g = m // 2
        wq_m = ptile(Wq[m * 256:(m + 1) * 256, :].T.astype(f))
        wk_m = ptile(Wk[g * 128:(g + 1) * 128, :].T.astype(f))
        wv_m = ptile(Wv[g * 128:(g + 1) * 128, :].T.astype(f))
        wo_m = ptile(Wo[:, m * 256:(m + 1) * 256].T.astype(f))
        bq_m = np.ascontiguousarray(
            (bq[m * 256:(m + 1) * 256] * scale).reshape(HL, D).T, dtype=f)
        bk_m = np.ascontiguousarray(bk[g * 128:(g + 1) * 128].reshape(D, 1),
                                    dtype=f)
        bv_m = np.ascontiguousarray(bv[g * 128:(g + 1) * 128].reshape(D, 1),
                                    dtype=f)
        in_maps.append({
            "xt": xT, "wq": wq_m, "wk": wk_m, "wv": wv_m, "wo": wo_m,
            "bq": bq_m, "bk": bk_m, "bv": bv_m,
            "cost": cosT, "sins": sins, "tri": tri, "ones": ones,
        })
    return in_maps


def kernel(x, cos, sin, Wq, bq, Wk, bk, Wv, bv, Wo, _trace=False):
    x, cos, sin = np.asarray(x), np.asarray(cos), np.asarray(sin)
    Wq, bq = np.asarray(Wq), np.asarray(bq)
    Wk, bk = np.asarray(Wk), np.asarray(bk)
    Wv, bv = np.asarray(Wv), np.asarray(bv)
    Wo = np.asarray(Wo)
    nc = _get_nc()
    in_maps = _prep_inputs(x, cos, sin, Wq, bq, Wk, bk, Wv, bv, Wo)
    res = run_bass_kernel_spmd(nc, in_maps, core_ids=list(range(NCORES)),
                               trace=_trace)
    out = res.results[0]["out"].astype(np.float64)
    for m in range(1, NCORES):
        out += res.results[m]["out"]
    out = out.astype(np.float32).reshape(B, T, C)
    if _trace:
        _CACHE["last_result"] = res
    return out


# revision 45
# speedup vs baseline: 1.0286x; 1.0286x over previous
"""Trainium2 Bass kernel: causal self-attention (GQA + RoPE) for
B=1, T=2048, C=2048, H=16 query heads, HKV=4 KV heads, D=128.

Sharding: tensor-parallel over heads across 8 NeuronCores. Core m computes
query heads {2m, 2m+1} and the single KV head (m//2) those heads attend to,
plus the o_proj partial product for its 256 input columns. The host sums the
8 partial outputs (the TP all-reduce).

Device-side layout (per core) is transposed so every matmul has its
contraction on the partition axis with no on-chip transposes of activations:
  qT[h]  = (Wq_h @ x.T + bq)/sqrt(D)   [D=128 part, T free]   (ACT eviction)
  kT     =  Wk_g @ x.T + bk            [128, T]
  vT     =  Wv_g @ x.T + bv            [128, T] -> PE-transposed to v [T,128]
  RoPE on qT/kT via sign-folded sin + partition-swap DMA copies.
  ST     = k'T.T @ q'T                 [tk 128-part, tq 512-free] per head
  P      = exp(ST) (no max subtraction: logits are bounded), causal mask via
           a triangular [128,128] multiply on diagonal tiles; fully-masked
           column ranges are skipped in the matmuls entirely.
  sums   = ones128.T @ P  (column sums broadcast across partitions, in PSUM)
  yT     = matmul(lhsT=v[tk,128], rhs=P) accumulated over tk
  y'T    = yT * reciprocal(sums)
  out    = y'T.T @ WoT (partial; host sums over cores)

Matmuls run as float32r (fp32 bits, reduced-precision PE mode, 1 cyc/row for
free dim >= 256 vs 4 for fp32). The BIR verifier requires every producer of
an fp32r matmul operand to emit fp32r, so those buffers are declared fp32r
end-to-end (DRAM and SBUF); engine output-casts do the advisory rounding.
"""

import math
import numpy as np
from contextlib import ExitStack

import concourse.bass as bass
import concourse.bacc as bacc
import concourse.tile as tile
from concourse import mybir
from concourse.bass_utils import run_bass_kernel_spmd
from concourse.masks import make_identity

B, T, C = 1, 2048, 2048
H, HKV = 16, 4
D = 128
NCORES = 8
HL = H // NCORES          # query heads per core
TQ = 512                  # query tile width (one fp32 PSUM bank)
NT = T // TQ              # 4 query tiles
NK = T // D               # 16 key tiles
NCT = C // 128            # 16 contraction tiles over the model dim
F32 = mybir.dt.float32
F32R = mybir.dt.float32r
Id = mybir.ActivationFunctionType.Identity
Exp = mybir.ActivationFunctionType.Exp

_CACHE: dict = {}


def _build():
    nc = bacc.Bacc(None, target_bir_lowering=False, debug=False)
    scale = 1.0 / math.sqrt(D)
    with tile.TileContext(nc) as tc, ExitStack() as ctx:
        dram = ctx.enter_context(tc.tile_pool(name="dram", bufs=1, space="DRAM"))

        def din(name, shape, dt=F32R):
            return dram.tile(shape, dt, kind="ExternalInput", name=name,
                             uniquify=False)

        xt_d = din("xt", [C, T])          # x[0].T
        # weights arrive host-pre-tiled to the SBUF layout [128, ...] so
        # each loads in one dma_start with 8-16KB per-partition descriptors
        wq_d = din("wq", [128, NCT * HL * D])
        wk_d = din("wk", [128, NCT * D])
        wv_d = din("wv", [128, NCT * D])
        wo_d = din("wo", [128, HL * C])
        tri_d = din("tri", [D, D])        # tri[i,j] = 1.0 if i<=j else 0.0
        one_d = din("ones", [D, D])
        bq_d = din("bq", [D, HL], F32)    # pre-scaled by 1/sqrt(D)
        bk_d = din("bk", [D, 1], F32)
        bv_d = din("bv", [D, 1], F32)
        cos_d = din("cost", [D, T], F32)  # cos[0].T
        sin_d = din("sins", [D, T], F32)  # sin[0].T with rows 0:64 negated
        out_d = dram.tile([T, C], mybir.dt.bfloat16, kind="ExternalOutput",
                          name="out", uniquify=False)

        const = ctx.enter_context(tc.tile_pool(name="const", bufs=1))
        wq_s = const.tile([128, NCT * HL * D], F32R, name="wq_s")
        wk_s = const.tile([128, NCT * D], F32R, name="wk_s")
        wv_s = const.tile([128, NCT * D], F32R, name="wv_s")
        # Weight loads go on the GpSimd SWDGE queue: the Sync sequencer
        # issues each dma_start serially (~700ns apiece), and it is needed
        # for the latency-critical x/out streams.
        nc.gpsimd.dma_start(out=wk_s[:], in_=wk_d[:])
        nc.gpsimd.dma_start(out=wv_s[:], in_=wv_d[:])
        nc.gpsimd.dma_start(out=wq_s[:], in_=wq_d[:])
        bq_s = const.tile([128, HL], F32, name="bq_s")
        bk_s = const.tile([128, 1], F32, name="bk_s")
        bv_s = const.tile([128, 1], F32, name="bv_s")
        cos_s = const.tile([128, T], F32, name="cos_s")
        sin_s = const.tile([128, T], F32, name="sin_s")
        tri_s = const.tile([128, 128], F32R, name="tri_s")
        ones_s = const.tile([128, 128], F32R, name="ones_s")
        ident_s = const.tile([128, 128], F32, name="ident_s")
        for dst, src in ((bq_s, bq_d), (bk_s, bk_d), (bv_s, bv_d),
                         (cos_s, cos_d), (sin_s, sin_d), (tri_s, tri_d),
                         (ones_s, one_d)):
            nc.sync.dma_start(out=dst[:], in_=src[:])
        make_identity(nc, ident_s[:])
        wo_s = const.tile([128, HL * C], F32R, name="wo_s")

        act = ctx.enter_context(tc.tile_pool(name="act", bufs=1))
        qr = [act.tile([128, T], F32R, name=f"qr{h}_s") for h in range(HL)]
        kr_s = act.tile([128, T], F32R, name="kr_s")
        vT_s = act.tile([128, T], F32, name="vT_s")
        v_s = act.tile([128, NK * D], F32R, name="v_s")
        ys = [act.tile([128, T], F32R, name=f"y{h}_s") for h in range(HL)]

        xpool = ctx.enter_context(tc.tile_pool(name="xpool", bufs=6))
        prepool = ctx.enter_context(tc.tile_pool(name="prepool", bufs=3))
        rpool = ctx.enter_context(tc.tile_pool(name="rpool", bufs=3))
        ppool = ctx.enter_context(tc.tile_pool(name="ppool", bufs=4))
        opool = ctx.enter_context(tc.tile_pool(name="opool", bufs=3))

        def rope(dst, pre, t):
            """dst = pre*cos + rot_half(pre)*sin on columns [t*TQ, (t+1)*TQ).

            dst is fp32r; the final add casts. All DVE inputs stay fp32.
            The partition-swap copies ride the GpSimd SWDGE queue.
            """
            sl = bass.ts(t, TQ)
            rot = rpool.tile([128, TQ], F32, tag="rot")
            nc.gpsimd.dma_start(out=rot[0:64, :], in_=pre[64:128, :])
            nc.gpsimd.dma_start(out=rot[64:128, :], in_=pre[0:64, :])
            nc.vector.tensor_mul(rot[:], rot[:], sin_s[:, sl])
            tmp = rpool.tile([128, TQ], F32, tag="rtmp")
            nc.vector.tensor_mul(tmp[:], pre[:], cos_s[:, sl])
            nc.vector.tensor_add(dst, tmp[:], rot[:])

        # ---- interleaved projection-pair / attention blocks --------------
        # t-pair blocks: one [128, 1024] x load feeds both halves' psum
        # accumulators (8 banks), halving dma_start count and giving 4KB
        # DMA lines. Attention for tq in {2tp, 2tp+1} only needs
        # projections <= the pair, so each pair's attention+o_proj runs
        # right after it: PE never crosses an idle phase boundary and the
        # output DMA stream starts halfway through the kernel.
        def proj_pair(tp, pa):
            ps = []  # [half][q0, q1, k, v]
            for half in range(2):
                ps.append([pa.tile([128, TQ], F32, tag=f"pp{half}{j}",
                                   name=f"pp{half}{j}")
                           for j in range(4)])
            for c in range(NCT):
                xt = xpool.tile([128, 2 * TQ], F32R, tag="x")
                nc.sync.dma_start(
                    out=xt[:],
                    in_=xt_d[bass.ts(c, 128), tp * 2 * TQ:(tp + 1) * 2 * TQ])
                st, sp = (c == 0), (c == NCT - 1)
                base = c * HL * D
                for half in range(2):
                    xh = xt[:, bass.ts(half, TQ)]
                    nc.tensor.matmul(ps[half][0][:], wq_s[:, base:base + D],
                                     xh, start=st, stop=sp)
                    nc.tensor.matmul(ps[half][1][:],
                                     wq_s[:, base + D:base + 2 * D],
                                     xh, start=st, stop=sp)
                    nc.tensor.matmul(ps[half][2][:], wk_s[:, bass.ts(c, D)],
                                     xh, start=st, stop=sp)
                    nc.tensor.matmul(ps[half][3][:], wv_s[:, bass.ts(c, D)],
                                     xh, start=st, stop=sp)
            # PSUM->SBUF evictions with fused bias (and 1/sqrt(D) on q),
            # split across ACT and DVE so the pair-boundary drains fast
            for half in range(2):
                t = 2 * tp + half
                sl = bass.ts(t, TQ)
                pre0 = prepool.tile([128, TQ], F32, tag="pre0")
                pre1 = prepool.tile([128, TQ], F32, tag="pre1")
                prek = prepool.tile([128, TQ], F32, tag="prek")
                nc.vector.tensor_scalar(pre0[:], ps[half][0][:], scale,
                                        bq_s[:, 0:1], mybir.AluOpType.mult,
                                        mybir.AluOpType.add)
                nc.vector.tensor_scalar(pre1[:], ps[half][1][:], scale,
                                        bq_s[:, 1:2], mybir.AluOpType.mult,
                                        mybir.AluOpType.add)
                nc.scalar.activation(prek[:], ps[half][2][:], Id,
                                     bias=bk_s[:, 0:1])
                nc.scalar.activation(vT_s[:, sl], ps[half][3][:], Id,
                                     bias=bv_s[:, 0:1])
                rope(qr[0][:, sl], pre0, t)
                rope(qr[1][:, sl], pre1, t)
                rope(kr_s[:, sl], prek, t)

        def vtrans(tp, vt):
            # transpose the pair's 8 new v tiles to natural layout; evict on
            # ACT (idle during phase A while DVE runs RoPE)
            for tk in range(8 * tp, 8 * tp + 8):
                vtp = vt.tile([128, D], F32, tag="vtp", name="vtp")
                nc.tensor.transpose(vtp[:], vT_s[:, bass.ts(tk, D)],
                                    ident_s[:])
                nc.scalar.copy(v_s[:, bass.ts(tk, D)], vtp[:])

        def attn_block(tq, pb):
            ntk = 4 * tq + 4
            if True:
                for h in range(HL):
                    sump = pb.tile([128, TQ], F32, tag="sum", bufs=1)
                    yp = pb.tile([128, TQ], F32, tag="yav")
                    # software-pipelined: score matmul+exp one tile ahead of
                    # the consuming sum/AV matmuls so PE never waits on ACT
                    pts = {}
                    for tk in range(ntk + 1):
                        if tk < ntk:
                            r = max(tk * D - tq * TQ, 0)  # masked col prefix
                            sp_ = pb.tile([128, TQ], F32, tag="s")
                            nc.tensor.matmul(
                                sp_[:, r:], kr_s[:, bass.ts(tk, D)],
                                qr[h][:, tq * TQ + r:(tq + 1) * TQ],
                                start=True, stop=True)
                            pt = ppool.tile([128, TQ], F32R, tag="p")
                            nc.scalar.activation(pt[:, r:], sp_[:, r:], Exp)
                            if tk * D >= tq * TQ:  # diagonal: triangular mask
                                nc.vector.tensor_mul(pt[:, r:r + D],
                                                     pt[:, r:r + D], tri_s[:])
                            pts[tk] = (pt, r)
                        if tk >= 1:
                            pt, r = pts.pop(tk - 1)
                            st, sp2 = (tk - 1 == 0), (tk - 1 == ntk - 1)
                            nc.tensor.matmul(sump[:, r:], ones_s[:],
                                             pt[:, r:], start=st, stop=sp2)
                            nc.tensor.matmul(yp[:, r:],
                                             v_s[:, bass.ts(tk - 1, D)],
                                             pt[:, r:], start=st, stop=sp2)
                    rec = rpool.tile([128, TQ], F32, tag="rec")
                    nc.vector.reciprocal_approx_fast(rec[:], sump[:])
                    nc.vector.tensor_mul(ys[h][:, bass.ts(tq, TQ)], yp[:],
                                         rec[:])
                # o_proj for the 4 row-tiles of this tq block; evict the 4
                # column tiles into one wide bf16 tile -> single DMA with
                # 4KB lines per output row-block
                for tt in range(4):
                    t = 4 * tq + tt
                    wide = opool.tile([128, C], mybir.dt.bfloat16, tag="oev")
                    for n in range(NT):
                        op_ = pb.tile([128, TQ], F32, tag="o", bufs=3)
                        for h in range(HL):
                            nc.tensor.matmul(
                                op_[:], ys[h][:, bass.ts(t, D)],
                                wo_s[:, h * C + n * TQ:h * C + (n + 1) * TQ],
                                start=(h == 0), stop=(h == HL - 1))
                        # o evictions all on DVE: ACT is the exp engine and
                        # near-saturated during attention
                        nc.vector.tensor_copy(wide[:, bass.ts(n, TQ)],
                                              op_[:])
                    nc.sync.dma_start(out=out_d[bass.ts(t, D), :],
                                      in_=wide[:])

        with tc.tile_pool(name="pa_psum", bufs=1, space="PSUM") as pa:
            for tp in range(NT // 2):
                proj_pair(tp, pa)
        with tc.tile_pool(name="vt_psum", bufs=4, space="PSUM") as vt:
            for tp in range(NT // 2):
                vtrans(tp, vt)
        nc.gpsimd.dma_start(out=wo_s[:], in_=wo_d[:])
        with tc.tile_pool(name="pb_psum", bufs=2, space="PSUM") as pb:
            for tq in range(NT):
                attn_block(tq, pb)
    nc.compile()
    return nc


def _get_nc():
    if "nc" not in _CACHE:
        _CACHE["nc"] = _build()
    return _CACHE["nc"]


def _prep_inputs(x, cos, sin, Wq, bq, Wk, bk, Wv, bv, Wo):
    f = np.float32
    xT = np.ascontiguousarray(x[0].T, dtype=f)
    cosT = np.ascontiguousarray(cos[0].T, dtype=f)
    sinT = np.ascontiguousarray(sin[0].T, dtype=f)
    sins = np.concatenate([-sinT[:64], sinT[64:]], axis=0)
    sins = np.ascontiguousarray(sins, dtype=f)
    idx = np.arange(D)
    tri = (idx[:, None] <= idx[None, :]).astype(f)
    ones = np.ones((D, D), dtype=f)
    scale = np.float32(1.0 / math.sqrt(D))
    in_maps = []
    def ptile(a):
        """[K*128, N] -> partition-major [128, K*N] matching the SBUF tiles."""
        k = a.shape[0] // 128
        return np.ascontiguousarray(
            a.reshape(k, 128, a.shape[1]).transpose(1, 0, 2).reshape(128, -1),
            dtype=f)

    for m in range(NCORES):
        g = m // 2
        wq_m = ptile(Wq[m * 256:(m + 1) * 256, :].T.astype(f))
        wk_m = ptile(Wk[g * 128:(g + 1) * 128, :].T.astype(f))
        wv_m = ptile(Wv[g * 128:(g + 1) * 128, :].T.astype(f))
        wo_m = ptile(Wo[:, m * 256:(m + 1) * 256].T.astype(f))
        bq_m = np.ascontiguousarray(
            (bq[m * 256:(m + 1) * 256] * scale).reshape(HL, D).T, dtype=f)
        bk_m = np.ascontiguousarray(bk[g * 128:(g + 1) * 128].reshape(D, 1),
                                    dtype=f)
        bv_m = np.ascontiguousarray(bv[g * 128:(g + 1) * 128].reshape(D, 1),
                                    dtype=f)
        in_maps.append({
            "xt": xT, "wq": wq_m, "wk": wk_m, "wv": wv_m, "wo": wo_m,
            "bq": bq_m, "bk": bk_m, "bv": bv_m,
            "cost": cosT, "sins": sins, "tri": tri, "ones": ones,
        })
    return in_maps


def kernel(x, cos, sin, Wq, bq, Wk, bk, Wv, bv, Wo, _trace=False):
    x, cos, sin = np.asarray(x), np.asarray(cos), np.asarray(sin)
    Wq, bq = np.asarray(Wq), np.asarray(bq)
    Wk, bk = np.asarray(Wk), np.asarray(bk)
    Wv, bv = np.asarray(Wv), np.asarray(bv)
    Wo = np.asarray(Wo)
    nc = _get_nc()
    in_maps = _prep_inputs(x, cos, sin, Wq, bq, Wk, bk, Wv, bv, Wo)
    res = run_bass_kernel_spmd(nc, in_maps, core_ids=list(range(NCORES)),
                               trace=_trace)
    out = res.results[0]["out"].astype(np.float64)
    for m in range(1, NCORES):
        out += res.results[m]["out"]
    out = out.astype(np.float32).reshape(B, T, C)
    if _trace:
        _CACHE["last_result"] = res
    return out
